# revision 68
# baseline (speedup 1.0000x reference)
"""Multi-head attention (q/k/v projections + softmax attention + out-projection)
on 8 Trainium2 NeuronCores.

Sharding: 16 (batch, head) units over 8 cores -> core c handles batch n = c//4
and head pair hp = c%4 (columns 128*hp : 128*hp+128 of the projections).
Per-core partial outputs (each pair's contribution to mix @ Wo) are summed on
host per batch, + bo.

Device kernel (per core), v2 (thin-PV rewrite; cost-model timeline 287us vs
the 418us v1 baseline; HW rel err 1.03e-2):
  - Inputs x (=qT/kT/vT [512,4096]) and all weights are bf16 (halves the DMA
    stream on the serialized DMA device); kpt/qpt kept f32r for score accuracy.
  - S^T[lkv,lq] = kpt_h.T @ qpt_h (f32r, [128,1024] psum tiles st0/st1), exp
    on ScalarE -> est bf16 in SBUF. ScalarE exp (~235us busy) is the critical
    engine; everything else hides under it.
  - 1/8 of the exp tiles run on DVE instead, as a Schraudolph approximation:
    int16(2^7/ln2 * x + b) bitcast as bf16 ~= exp(x) (rms 1.8%, mean bias
    cancels in softmax). Placed on j's where DVE is idle; heads balanced.
  - PV is FLIPPED: est slice [128lkv,128lq] is the stationary matrix, vp chunk
    [128lkv, 65] bf16 (64 v-cols + ones col) is the moving one -> psum
    [128lq, 65] accumulated over the 32 kv chunks, emitted one j behind the
    st/exp stream. Cost-wise this streams 65 cols instead of 1024, halving PE
    attention time; the ones col accumulates sumexp into each group's col 64.
  - pv psum packing: 8 lq-tiles x (65+65) cols = 1040 cols in 3 banks
    (3+3+2 groups). Only the first matmul into a bank uses start=True (psum
    zero regions are whole banks); other groups rely on the lazy
    pending-zero, with skip_group_check.
  - Post-lqc: recip of the sumexp cols + normalize into mixn bf16 [128lq,
    128c] (DVE; GPSIMD cannot read PSUM), PE-transpose via a bitcast corner
    of the op bank -> mixT [128c, 128lq], then ONE out-proj matmul per
    lq-tile (both heads contract together), copy, DMA out. No DRAM bounce
    for sumexp, no per-head scale+add. The per-tile chains are spread one
    per j over the next lqc's stream; the lqc3 tail runs depth-first through
    all four free psum banks with the idle Act engine helping the copies.
  - lqc0 is FUSED with the projections: exp results for j<16 go to an SBUF
    stash (est consumers deferred), freeing the pv banks so K/V/Q chunk
    projections can use them as scratch; the PV backlog drains interleaved
    over j=16..27. K chunk 0 is projected in two half-width pieces off a
    split DMA so the first exp starts ~8us in. Input-chunk DMAs stream JIT
    on 2 queues (SP/HWDGE: wkq, xk*, xq*, out; Pool/SWDGE: wv, xv*) with
    the PE p-state warmed by dummy matmuls.
"""

import numpy as np
import ml_dtypes

import concourse.bacc as bacc
import concourse.mybir as mybir
import concourse.tile as tile
from concourse import bass_utils, masks

P = 128
L = 4096
D = 512
F32 = mybir.dt.float32
F32R = mybir.dt.float32r
BF16 = mybir.dt.bfloat16
I16 = mybir.dt.int16
AF = mybir.ActivationFunctionType
MUL = mybir.AluOpType.mult
ADD = mybir.AluOpType.add

# Schraudolph exp in the bf16 domain: int16(A16*x + B16) bitcast as bf16
# approximates exp(x) (max rel ~3.3%, rms ~1.8%; the mean bias cancels in
# softmax). Used on DVE for a subset of tiles to offload the Act engine.
A16 = float(2**7 / np.log(2))
B16 = float(127 * 128 - 5.5)

_NC = None

NSTASH = 16  # j-iterations of lqc0 whose est goes to the stash


def build():
    nc = bacc.Bacc("TRN2", target_bir_lowering=False, debug=False)

    xqt = nc.dram_tensor("xqt", (D, L), BF16, kind="ExternalInput").ap()
    xkt = nc.dram_tensor("xkt", (D, L), BF16, kind="ExternalInput").ap()
    xvt = nc.dram_tensor("xvt", (D, L), BF16, kind="ExternalInput").ap()
    # wkq packs wk and wq as [128, 4, 256] (p o m layout, wk cols 0:128,
    # wq cols 128:256) so one DMA with 512 descriptors loads both.
    wkq = nc.dram_tensor("wkq", (P, 4, 2 * P), BF16, kind="ExternalInput").ap()
    wv = nc.dram_tensor("wv", (D, P), BF16, kind="ExternalInput").ap()
    wo = nc.dram_tensor("wo", (P, D), BF16, kind="ExternalInput").ap()
    bqs = nc.dram_tensor("bqs", (P, 1), F32, kind="ExternalInput").ap()
    bkc = nc.dram_tensor("bkc", (P, 1), F32, kind="ExternalInput").ap()
    bvr = nc.dram_tensor("bvr", (1, P), BF16, kind="ExternalInput").ap()
    # per-core partials leave in bf16 (halves the output DMA stream and the
    # tail drain); the host upcasts and sums — quantization of partials adds
    # only ~1e-3 to the max-normalized error
    out = nc.dram_tensor("out", (L, D), BF16, kind="ExternalOutput").ap()

    with tile.TileContext(nc) as tc:
        with tc.tile_pool(name="const", bufs=1) as const, \
             tc.tile_pool(name="persist", bufs=1) as persist, \
             tc.tile_pool(name="xs", bufs=2) as xs, \
             tc.tile_pool(name="psp", bufs=1, space="PSUM") as psp, \
             tc.tile_pool(name="esp", bufs=3) as esp, \
             tc.tile_pool(name="smallp", bufs=8) as smallp, \
             tc.tile_pool(name="mixp", bufs=4) as mixp, \
             tc.tile_pool(name="outp", bufs=4) as outp:
            # ---- queue layout ----
            # SP (HWDGE): xq0, xq1, xk0..7, xq2..7, then output DMAs.
            # Pool (SWDGE): weights/consts (+ memsets, identity), xv0..7.
            # First-exp critical path: xq0 (SP, immediate) || wq (Pool,
            # immediate) -> Qproj0 -> Kproj0 -> st(0,0) -> exp.
            xqv = xqt.rearrange("(o p) l -> p o l", p=P)
            xkv = xkt.rearrange("(o p) l -> p o l", p=P)
            xvv = xvt.rearrange("(o p) l -> p o l", p=P)

            xq_tiles = {}
            xk_tiles = {}
            xv_tiles = {}
            # Gating weights first on SP (they must beat the x stream on the
            # serialized DMA device), then xk0 (K-proj gates the first st),
            # then xq0/xq1, then the rest of the streams.
            wkq_sb = const.tile([P, 4, 2 * P], BF16, tag="wkq")
            nc.sync.dma_start(wkq_sb, wkq)
            wk_sb = wkq_sb[:, :, 0:P]
            wq_sb = wkq_sb[:, :, P:2 * P]
            # K chunk 0 split in two half-width pieces: the first st only
            # needs kpt0 cols 0:256, so piece A's short DMA unblocks it early
            xk0a = xs.tile([P, 4, 256], BF16, tag="xk0a", bufs=1,
                           name="xk0a")
            nc.sync.dma_start(xk0a, xkv[:, :, 0:256])
            t = xs.tile([P, 4, 512], BF16, tag="xq0", name="xq0")
            nc.sync.dma_start(t, xqv[:, :, 0:512])
            xq_tiles[0] = t
            bk_sb = const.tile([P, 1], F32, tag="bk")
            nc.sync.dma_start(bk_sb, bkc)
            bq_sb = const.tile([P, 1], F32, tag="bq")
            nc.sync.dma_start(bq_sb, bqs)
            t = xs.tile([P, 4, 512], BF16, tag="xq1", name="xq1")
            nc.sync.dma_start(t, xqv[:, :, 512:1024])
            xq_tiles[1] = t
            xk0b = xs.tile([P, 4, 256], BF16, tag="xk0b", bufs=1,
                           name="xk0b")
            nc.sync.dma_start(xk0b, xkv[:, :, 256:512])
            for ch in range(1, 8):
                t = xs.tile([P, 4, 512], BF16, tag=f"xk{ch % 2}", bufs=3,
                            name=f"xk{ch}")
                nc.sync.dma_start(t, xkv[:, :, ch * 512:(ch + 1) * 512])
                xk_tiles[ch] = t
            for ch in range(2, 8):
                t = xs.tile([P, 4, 512], BF16, tag=f"xq{ch % 2}",
                            name=f"xq{ch}")
                nc.sync.dma_start(t, xqv[:, :, ch * 512:(ch + 1) * 512])
                xq_tiles[ch] = t

            kpt_t = [persist.tile([P, 512], F32R, tag=f"kpt{c}",
                                  name=f"kpt{c}") for c in range(8)]
            qpt_t = [persist.tile([P, 512], F32R, tag=f"qpt{c}",
                                  name=f"qpt{c}") for c in range(8)]
            vp_t = [persist.tile([P, 4, 130], BF16, tag=f"vp{c}",
                                 name=f"vp{c}") for c in range(8)]
            onesr = const.tile([1, P], BF16, tag="onesr")
            nc.gpsimd.memset(onesr, 1.0)
            for c in range(8):
                # ones cols at 64 and 129 of each js group
                nc.gpsimd.memset(vp_t[c][:, :, 64::65], 1.0)

            es_t = [persist.tile([P, 1024], BF16, tag=f"es{i}",
                                 name=f"es{i}") for i in range(2 * NSTASH)]

            wv_sb = const.tile([P, 4, P], BF16, tag="wv")
            nc.gpsimd.dma_start(wv_sb, wv.rearrange("(o p) m -> p o m", p=P))
            bvr_sb = const.tile([1, P], BF16, tag="bvr")
            nc.gpsimd.dma_start(bvr_sb, bvr)
            for ch in range(8):
                t = xs.tile([P, 4, 512], BF16, tag=f"xv{ch % 2}",
                            name=f"xv{ch}")
                nc.gpsimd.dma_start(t, xvv[:, :, ch * 512:(ch + 1) * 512])
                xv_tiles[ch] = t
            wo_sb = const.tile([P, D], BF16, tag="wo")
            nc.gpsimd.dma_start(wo_sb, wo)
            ident = const.tile([P, P], BF16, tag="ident")
            masks.make_identity(nc, ident)

            # ---- projection emitters (psum scratch: pv0/pv1/pv2 tags) ----
            def proj_q(ch, ptag, act_copy=False):
                ps = psp.tile([P, 512], F32, tag=ptag, name=f"qps{ch}")
                xt = xq_tiles[ch]
                for dk in range(4):
                    nc.tensor.matmul(ps, lhsT=wq_sb[:, dk, :],
                                     rhs=xt[:, dk, :],
                                     start=(dk == 0), stop=(dk == 3))
                if act_copy:
                    # prologue: Act is idle and its next exp waits on this
                    # copy anyway — doing it on Act removes the DVE hop
                    nc.scalar.activation(qpt_t[ch][:], ps, AF.Identity,
                                         bias=bq_sb, scale=0.125)
                else:
                    nc.vector.tensor_scalar(qpt_t[ch][:], ps, 0.125, bq_sb,
                                            MUL, ADD)

            def proj_k(ch, ptag):
                ps = psp.tile([P, 512], F32, tag=ptag, name=f"kps{ch}")
                xt = xk_tiles[ch]
                for dk in range(4):
                    nc.tensor.matmul(ps, lhsT=wk_sb[:, dk, :],
                                     rhs=xt[:, dk, :],
                                     start=(dk == 0), stop=(dk == 3))
                nc.vector.tensor_scalar(kpt_t[ch][:], ps, 1.0, bk_sb,
                                        MUL, ADD)

            def proj_v(ch, ptag):
                # out: [lkv(4x128), c(128)] in one bank (4 groups of 128 cols)
                ps = psp.tile([P, 512], F32, tag=ptag, name=f"vps{ch}")
                xt = xv_tiles[ch]
                for js in range(4):
                    sl = ps[:, js * P:(js + 1) * P]
                    for dk in range(4):
                        nc.tensor.matmul(
                            sl, lhsT=xt[:, dk, js * P:(js + 1) * P],
                            rhs=wv_sb[:, dk, :],
                            start=(js == 0 and dk == 0), stop=False,
                            skip_group_check=True)
                    nc.tensor.matmul(sl, lhsT=onesr, rhs=bvr_sb,
                                     start=False,
                                     stop=(js == 3),
                                     skip_group_check=True)
                # copy mix cols per head into vp (bf16), around the ones cols
                nc.vector.tensor_copy(vp_t[ch][:, :, 0:64],
                                      ps.rearrange("p (j c) -> p j c", j=4)
                                      [:, :, 0:64])
                nc.vector.tensor_copy(vp_t[ch][:, :, 65:129],
                                      ps.rearrange("p (j c) -> p j c", j=4)
                                      [:, :, 64:128])

            # fused-phase projection schedule (Q0,Q1,K0 are the prologue):
            # K(ch) at j=ch+1 (hard deadline j=4ch), V(ch) at j=ch+9 (all
            # vp needed when the PV backlog starts), Q2/Q3 via the op bank
            # at j=17/19 (needed at lqc1), Q4..Q7 deferred into the lqc1/2
            # j-loops (op bank is idle there after j=11).
            sched = {}
            for ch in range(1, 8):
                sched.setdefault(ch + 1, []).append(("k", ch))
            for ch in range(8):
                sched.setdefault(ch + 9, []).append(("v", ch))
            sched.setdefault(17, []).append(("q", 2))
            sched.setdefault(19, []).append(("q", 3))
            ptags = ["pv0", "pv1", "pv2"]

            # PE p-state warmup: keep the PE busy from ~0.4us until the
            # first projection (~5.5us) so projections run at full clock.
            warm = psp.tile([1, P], F32, tag="op", name="warm")
            for i in range(22):
                nc.tensor.matmul(warm, lhsT=onesr[:, 0:1], rhs=onesr,
                                 start=(i == 0), stop=(i == 21),
                                 skip_group_check=True)

            # prologue projections; K0 in two half-width pieces (each piece
            # is its own accumulation group in the same bank — lazy zero)
            kps0 = psp.tile([P, 512], F32, tag="pv2", name="kps0")
            for piece, xt in ((0, xk0a), (1, xk0b)):
                sl = kps0[:, piece * 256:(piece + 1) * 256]
                for dk in range(4):
                    nc.tensor.matmul(
                        sl, lhsT=wk_sb[:, dk, :],
                        rhs=xt[:, dk, :],
                        start=(piece == 0 and dk == 0),
                        stop=(piece == 1 and dk == 3),
                        skip_group_check=True)
                nc.vector.tensor_scalar(
                    kpt_t[0][:, piece * 256:(piece + 1) * 256], sl,
                    1.0, bk_sb, MUL, ADD)
                if piece == 0:
                    proj_q(0, "pv0")
            proj_q(1, "pv1")
            task_i = 1

            # ---- attention ----
            # pv bank layout: bank b=t//3 (tags pv0,pv1,pv2), slot s=t%3,
            # group (t,h) cols 130*s+65*h .. +65. First matmul per bank
            # start=True; everything else start=False (lazy zero region).
            def emit_st_exp(lqc, j, est_dst, split=False):
                """S matmuls + exp for both heads of (lqc, j).
                est_dst: (tile_h0, tile_h1). split=True issues a 512-wide
                exp right after each hf matmul, hf-major so the first four
                half-exps depend only on qpt[2lqc] (prologue fill)."""
                sts = [psp.tile([P, 1024], F32, tag=f"st{h}",
                                name=f"st_{lqc}_{j}_{h}") for h in range(2)]

                def mm(h, hf):
                    nc.tensor.matmul(
                        sts[h][:, hf * 512:(hf + 1) * 512],
                        lhsT=kpt_t[j // 4][h * 64:h * 64 + 64,
                                           (j % 4) * P:(j % 4 + 1) * P],
                        rhs=qpt_t[2 * lqc + hf][h * 64:h * 64 + 64, :],
                        start=True, stop=True)

                if split:
                    for hf in range(2):
                        for h in range(2):
                            mm(h, hf)
                            nc.scalar.activation(
                                est_dst[h][:, hf * 512:(hf + 1) * 512],
                                sts[h][:, hf * 512:(hf + 1) * 512], AF.Exp)
                else:
                    # offload some tiles to a DVE Schraudolph exp to
                    # unload the Act engine — placed on j's where DVE
                    # is otherwise idle (not during post_out j=4..11,
                    # nor lqc0's projection phase, nor the tail of
                    # lqc3 where a slow last exp would delay pv(31))
                    lo, hi = (17, 31) if lqc == 0 else (
                        (13, 27) if lqc < 3 else (9, 23))
                    for h in range(2):
                        mm(h, 0)
                        mm(h, 1)
                        if (lo <= j <= hi and j % 2 == 1
                                and h == (j // 2) % 2):
                            nc.vector.tensor_scalar(
                                est_dst[h].bitcast(I16), sts[h],
                                A16, B16, MUL, ADD)
                        else:
                            nc.scalar.activation(est_dst[h], sts[h], AF.Exp)

            def emit_pv(pv_tiles, bank_started, j, est_pair, last=False):
                vch, vjs = j // 4, j % 4
                for t in range(8):
                    b, s = t // 3, t % 3
                    for h in range(2):
                        first = not bank_started[b]
                        bank_started[b] = True
                        nc.tensor.matmul(
                            pv_tiles[b][:, 130 * s + 65 * h:
                                        130 * s + 65 * h + 65],
                            lhsT=est_pair[h][:, t * P:(t + 1) * P],
                            rhs=vp_t[vch][:, vjs, 65 * h:65 * h + 65],
                            start=first, stop=last and t == 7 and h == 1,
                            skip_group_check=True)

            def post_norm_tile(lqc, pv_tiles, t, h1_act=False):
                """Recip + normalized mixn for one lq-tile of lqc. GPSIMD
                cannot touch PSUM, so both norms run on DVE; at the tail
                the idle Act engine takes h1 (Copy with an AP scale)."""
                b, s = t // 3, t % 3
                rT = smallp.tile([P, 2], F32, tag=f"rT{t}",
                                 name=f"rT{lqc}_{t}")
                sums = pv_tiles[b][:, 130 * s + 64::65][:, 0:2]
                nc.vector.reciprocal(rT, sums)
                mixn = smallp.tile([P, P], BF16, tag=f"mixn{t}",
                                   name=f"mixn{lqc}_{t}")
                nc.vector.tensor_scalar_mul(
                    mixn[:, 0:64],
                    pv_tiles[b][:, 130 * s:130 * s + 64],
                    rT[:, 0:1])
                h1_in = pv_tiles[b][:, 130 * s + 65:130 * s + 129]
                if h1_act:
                    nc.scalar.activation(mixn[:, 64:128], h1_in,
                                         AF.Copy, scale=rT[:, 1:2])
                else:
                    nc.vector.tensor_scalar_mul(mixn[:, 64:128], h1_in,
                                                rT[:, 1:2])
                return mixn

            def post_norm(lqc, pv_tiles):
                """All 8 lq-tiles breadth-first (frees pv banks early for
                the next lqc's accumulation)."""
                return [post_norm_tile(lqc, pv_tiles, t) for t in range(8)]

            def post_out(lqc, mixns, t, optag="op", ob_eng=None):
                """Transpose + out-proj + store for one lq-tile of lqc."""
                q0 = lqc * 1024
                op = psp.tile([P, 512], F32, tag=optag,
                              name=f"op{lqc}_{t}")
                tp = op[:, 0:64].bitcast(BF16)
                nc.tensor.transpose(tp, mixns[t], ident)
                mixT = mixp.tile([P, P], BF16, tag="mixT",
                                 name=f"mixT{lqc}_{t}")
                nc.vector.tensor_copy(mixT, tp)
                op2 = psp.tile([P, 512], F32, tag=optag,
                               name=f"op2_{lqc}_{t}")
                nc.tensor.matmul(op2, lhsT=mixT, rhs=wo_sb,
                                 start=True, stop=True)
                ob = outp.tile([P, 512], BF16, tag="ob",
                               name=f"ob{lqc}_{t}")
                if ob_eng is nc.scalar:
                    nc.scalar.copy(ob, op2)
                else:
                    nc.vector.tensor_copy(ob, op2)
                nc.sync.dma_start(
                    out[q0 + t * P:q0 + (t + 1) * P, :], ob)

            # ---------------- lqc0 (fused with projections) ----------------
            pv_tiles = [psp.tile([P, 512], F32, tag=ptags[b],
                                 name=f"pv0_{b}") for b in range(3)]
            bank_started = [False, False, False]
            backlog = list(range(NSTASH))
            pend = None

            def run_tasks(j):
                nonlocal task_i
                for kind, ch in sched.get(j, []):
                    if kind == "k":
                        proj_k(ch, ptags[task_i % 3])
                    elif kind == "v":
                        proj_v(ch, ptags[task_i % 3])
                    else:
                        proj_q(ch, "op")
                    task_i += 1

            for j in range(32):
                if j < NSTASH:
                    emit_st_exp(0, j, (es_t[2 * j], es_t[2 * j + 1]),
                                split=(j < 2))
                    run_tasks(j)
                else:
                    e0 = esp.tile([P, 1024], BF16, tag="e0", name=f"e0_{j}")
                    e1 = esp.tile([P, 1024], BF16, tag="e1", name=f"e1_{j}")
                    emit_st_exp(0, j, (e0, e1))
                    run_tasks(j)
                    # drain backlog: 1/j on projection j's, 2 otherwise
                    nb = min(len(backlog), 1 if sched.get(j) else 2)
                    for _ in range(nb):
                        bj = backlog.pop(0)
                        emit_pv(pv_tiles, bank_started, bj,
                                (es_t[2 * bj], es_t[2 * bj + 1]))
                    # pipeline: pv runs one slot behind st/exp so PE never
                    # queues behind the (possibly slow DVE) exp of this j
                    if pend is not None:
                        emit_pv(pv_tiles, bank_started, *pend)
                    pend = (j, (e0, e1))
            assert not backlog, f"backlog not drained: {backlog}"

            # ---------------- lqc 1..3 ----------------
            # Boundary pipelining: run 4 st/exp pairs of the new lqc ahead
            # (est bufs=4) before draining the previous lqc's pv banks; the
            # transpose/out-proj/store of the previous lqc is spread one
            # lq-tile per j over j=4..11 so it never head-of-line blocks
            # the attention stream.
            # Boundary heads reuse the (long-dead) lqc0 est-stash tiles, so
            # the lookahead depth costs no extra SBUF.
            BDEPTH = 6
            for lqc in range(1, 4):
                prev_pv = pv_tiles
                prev_started = bank_started
                heads = []
                for j in range(2):
                    emit_st_exp(lqc, j, (es_t[2 * j], es_t[2 * j + 1]))
                    heads.append((es_t[2 * j], es_t[2 * j + 1]))
                # previous lqc's pending pv(31) closes its accumulation
                emit_pv(prev_pv, prev_started, *pend, last=True)
                for j in range(2, BDEPTH):
                    emit_st_exp(lqc, j, (es_t[2 * j], es_t[2 * j + 1]))
                    heads.append((es_t[2 * j], es_t[2 * j + 1]))
                mixns = post_norm(lqc - 1, prev_pv)
                pv_tiles = [psp.tile([P, 512], F32, tag=ptags[b],
                                     name=f"pv{lqc}_{b}") for b in range(3)]
                bank_started = [False, False, False]
                for j in range(BDEPTH - 1):
                    emit_pv(pv_tiles, bank_started, j, heads[j])
                pend = (BDEPTH - 1, heads[BDEPTH - 1])
                for j in range(BDEPTH, 32):
                    e0 = esp.tile([P, 1024], BF16, tag="e0",
                                  name=f"e0_{lqc}_{j}")
                    e1 = esp.tile([P, 1024], BF16, tag="e1",
                                  name=f"e1_{lqc}_{j}")
                    emit_st_exp(lqc, j, (e0, e1))
                    if BDEPTH <= j < BDEPTH + 8:
                        post_out(lqc - 1, mixns, j - BDEPTH)
                    # deferred Q projections (op bank is idle after j=11):
                    # qpt4/5 during lqc1, qpt6/7 during lqc2
                    if lqc <= 2 and j in (13, 15):
                        proj_q(2 * lqc + 2 + (j - 13) // 2, "op")
                    emit_pv(pv_tiles, bank_started, *pend)
                    pend = (j, (e0, e1))
            # tail: close lqc3, then drain depth-first per lq-tile (first
            # output DMA starts as early as possible), rotating the op work
            # through all four free psum banks and both copy engines
            emit_pv(pv_tiles, bank_started, *pend, last=True)
            # norms run 3 tiles ahead of the out chains; a pv bank only
            # becomes an op-chain target after all its tiles are normalized
            # (pv0 after norm 2, pv1 after norm 5, pv2 after norm 7)
            mixns = [post_norm_tile(3, pv_tiles, t, h1_act=True)
                     for t in range(3)]
            tail_tags = ["op", "pv0", "op", "pv0", "pv1", "pv2", "pv1", "pv2"]
            for t in range(8):
                if t + 3 < 8:
                    mixns.append(post_norm_tile(3, pv_tiles, t + 3,
                                                h1_act=True))
                post_out(3, mixns, t, optag=tail_tags[t],
                         ob_eng=(nc.scalar if t % 2 == 0 else nc.vector))

    nc.compile()
    return nc


def get_nc():
    global _NC
    if _NC is None:
        _NC = build()
    return _NC


def make_in_maps(q, k, v, Wq, bq, Wk, bk, Wv, bv, Wo, bo):
    q = np.asarray(q, np.float32)
    k = np.asarray(k, np.float32)
    v = np.asarray(v, np.float32)
    Wq = np.asarray(Wq, np.float32)
    Wk = np.asarray(Wk, np.float32)
    Wv = np.asarray(Wv, np.float32)
    Wo = np.asarray(Wo, np.float32)
    bq = np.asarray(bq, np.float32)
    bk = np.asarray(bk, np.float32)
    bv = np.asarray(bv, np.float32)
    bf = ml_dtypes.bfloat16
    xts = {}
    for n in range(2):
        xts[n] = (np.ascontiguousarray(q[n].T).astype(bf),
                  np.ascontiguousarray(k[n].T).astype(bf),
                  np.ascontiguousarray(v[n].T).astype(bf))
    in_maps = []
    for c in range(8):
        n, hp = c // 4, c % 4
        sl = slice(P * hp, P * (hp + 1))
        xq, xk, xv = xts[n]
        # wkq: [p, o, 2*128] with wk at cols 0:128, wq at 128:256, where
        # row (o*128+p) of the [512,128] weight maps to [p, o, :]
        wk_r = Wk[:, sl].reshape(4, P, P).transpose(1, 0, 2)
        wq_r = Wq[:, sl].reshape(4, P, P).transpose(1, 0, 2)
        wkq_h = np.concatenate([wk_r, wq_r], axis=2)
        in_maps.append({
            "xqt": xq, "xkt": xk, "xvt": xv,
            "wkq": np.ascontiguousarray(wkq_h).astype(bf),
            "wv": np.ascontiguousarray(Wv[:, sl]).astype(bf),
            "wo": np.ascontiguousarray(Wo[sl, :]).astype(bf),
            "bqs": (bq[sl] * 0.125).reshape(P, 1).astype(np.float32),
            "bkc": bk[sl].reshape(P, 1).astype(np.float32),
            "bvr": bv[sl].reshape(1, P).astype(bf),
        })
    return in_maps


def assemble(results, bo):
    bo = np.asarray(bo, np.float32)
    out = np.zeros((2, L, D), np.float32)
    for c in range(8):
        out[c // 4] += np.asarray(results[c]["out"], np.float32)
    out += bo[None, None, :]
    return out


def kernel(q, k, v, Wq, bq, Wk, bk, Wv, bv, Wo, bo):
    nc = get_nc()
    in_maps = make_in_maps(q, k, v, Wq, bq, Wk, bk, Wv, bv, Wo, bo)
    res = bass_utils.run_bass_kernel_spmd(nc, in_maps, core_ids=list(range(8)))
    return assemble(res.results, bo)


if __name__ == "__main__":
    build()
    print("build ok")
